# revision 81
# baseline (speedup 1.0000x reference)
"""Trainium2 Bass kernel for EnhancedOFTQKVLayer.

Computes out[b,s,o] = x[b,s,:] @ filt[o,:]^T + bias[o] where
filt = [Wq @ BD(cayley(q_R)); Wk @ BD(cayley(k_R)); Wv @ BD(cayley(v_R))]
(BD = block-diagonal, cayley(A) = (I-S) inv(I+S+eps I), S = 0.5(A-A^T)).

Distribution: data-parallel — batch b (8 rows) sharded one per NeuronCore;
attn_weight / bias / rotation blocks replicated. Per core:
  1. Cayley via SPD Newton-Schulz on P = (1+eps)^2 I - S^2, in 3 wide sets
     of 8 blocks (halves the elementwise instruction count vs 6x4). Degree-2
     minimax init (X0 = aI + bP + cP^2) cuts the fp16 iterations to 6; one
     symmetrization right before the fp32 polish controls lhsT-asymmetry
     drift. Emission is stage-wise (all g1, all T1', all g2, all casts) so
     independent sets never queue behind dependent ops on the in-order
     engines; T1'/cast work alternates between the DVE and Act engines.
     The Q = B^T X stage stays fp32 end-to-end (the product has ~1e3x
     cancellation; fp16 there costs 7e-3 of relative error).
  2. W^T built by PE transposes straight into the FT tiles, interleaved
     into prep/Newton PE bubbles; phase B overwrites FT in place with
     Q^T W^T (no separate W^T staging buffers).
  3. Main matmul fp16 (fp32 PSUM): x tiles DMA -> fp16 cast (scalar) ->
     PE transpose -> xT; phase C is a pure 512-row matmul stream at the
     16-bit PE roofline (~215 ns per 128x128x512 matmul), og-outer so the
     PSUM evictions (DVE, +bias) trail each output group; out-DMAs split
     across both HWDGE queues.
"""

import numpy as np

import concourse.bass as bass
import concourse.mybir as mybir
import concourse.tile as tile
from concourse import bacc
from concourse.bass import ts
from concourse.masks import make_identity
from concourse.bass_utils import run_bass_kernel_spmd

F32 = mybir.dt.float32
F16 = mybir.dt.float16

MAIN_DT = F16            # dtype of the big matmul inputs (x, filtT)

HIDDEN = 1024
OUT_DIM = 3 * HIDDEN
SEQ = 4096
P = 128
NBLK = 8                 # 128-blocks per hidden
NROT = 24                # 3 * NBLK rotation blocks
EPS = 1e-6
N_CORES = 8

NSETS = 3                # Newton processes blocks in sets of 8
SETB = 8                 # (wide sets: half the elementwise instructions)

# Newton-Schulz schedule (validated offline vs the jax reference with the
# exact kernel arithmetic incl. lhsT transposes: max Q rel err 1.2e-4).
NEWTON_F16 = 6
SYM_ITERS = {5}          # symmetrize right before the fp32 polish
NEWTON_F32 = 1
# X0 = aI + bP + cP^2 (degree-2 minimax init on [1, 260], E=0.934)
X0_A = 0.06628076593502354
X0_B = -0.0006737693182296712
X0_C = 1.7209944271511398e-06

M_TILES = SEQ // P       # 32
O_TILES = OUT_DIM // 512  # 6
XT_BUFS = 10             # in-flight transposed x tiles
PREFETCH = 10            # x tiles prepped during phase A
LOOKAHEAD = 3            # x tiles prepped ahead inside phase C


def build_body(ctx, tc):
    nc = tc.nc
    _PS_A = []

    x = nc.dram_tensor("x", [SEQ, HIDDEN], F32, kind="ExternalInput").ap()
    w = nc.dram_tensor("w", [OUT_DIM, HIDDEN], F32, kind="ExternalInput").ap()
    bias = nc.dram_tensor("bias", [OUT_DIM], F32, kind="ExternalInput").ap()
    rmat = nc.dram_tensor("rmat", [NROT, P, P], F32, kind="ExternalInput").ap()
    out = nc.dram_tensor("out", [SEQ, OUT_DIM], F32, kind="ExternalOutput").ap()

    sub = nc.vector.tensor_sub
    add = nc.vector.tensor_add
    smul = nc.vector.tensor_scalar_mul
    cp = nc.vector.tensor_copy
    acp = nc.any.tensor_copy
    ACT_COPY = mybir.ActivationFunctionType.Copy

    def bc(t):  # broadcast a [P, P] constant over a set's middle dim
        return t[:].unsqueeze(1).to_broadcast([P, SETB, P])

    # ---- persistent pools ----
    const = ctx.enter_context(tc.tile_pool(name="const", bufs=1))
    ftp = ctx.enter_context(tc.tile_pool(name="ftp", bufs=1))
    xrp = ctx.enter_context(tc.tile_pool(name="xrp", bufs=2))
    xbp = ctx.enter_context(tc.tile_pool(name="xbp", bufs=2))
    xtp = ctx.enter_context(tc.tile_pool(name="xtp", bufs=XT_BUFS))
    obp = ctx.enter_context(tc.tile_pool(name="obp", bufs=4))
    ps_tp = ctx.enter_context(tc.tile_pool(name="ps_tp", bufs=2, space="PSUM"))

    ident32 = const.tile([P, P], F32)
    make_identity(nc, ident32)
    identb = const.tile([P, P], MAIN_DT)
    cp(identb[:], ident32[:])
    eI2 = const.tile([P, P], F32)       # (1+eps)^2 I
    smul(eI2[:], ident32[:], float((1.0 + EPS) ** 2))
    eI12 = const.tile([P, P], F32)      # ((1+eps) + (1+eps)^2) I
    smul(eI12[:], ident32[:], float((1.0 + EPS) + (1.0 + EPS) ** 2))
    twoI = const.tile([P, P], F32)      # 2 I
    smul(twoI[:], ident32[:], 2.0)
    aI0 = const.tile([P, P], F16)       # X0_A * I  (Newton init)
    smul(aI0[:], ident32[:], float(X0_A))
    two_eye16 = const.tile([P, P], F16)  # 2 I (fp16, Newton rhs)
    smul(two_eye16[:], ident32[:], 2.0)

    bias_bc = const.tile([P, OUT_DIM], MAIN_DT)
    with tc.tile_pool(name="biasld", bufs=1) as bl:
        brow = bl.tile([1, OUT_DIM], F32)
        nc.sync.dma_start(brow[:], bias.unsqueeze(0))
        cp(bias_bc[:1, :], brow[:])
    nc.gpsimd.partition_broadcast(bias_bc[:], bias_bc[:1, :])

    # filtT chunks: FT[og][c, k, o'] = filtT[k*128+c, og*512+o'].
    # First filled with W^T (streamed during Newton), then overwritten
    # in place by Q^T @ W^T in phase B.
    FT = [ftp.tile([P, NBLK, 512], MAIN_DT, tag=f"ft{og}", name=f"ft{og}")
          for og in range(O_TILES)]

    # x-tile prep. Phase C path (PE-bound there): fp16 cast (scalar) ->
    # fp16 PE transpose (1 cyc/row) -> copy to SBUF. Phase A path (PE has
    # slack, elementwise engines are the bottleneck): fp32 PE transpose
    # (2 cyc/row) -> single fused cast-eviction, no separate cast op.
    def emit_xprep(mt, in_a=False):
        xr = xrp.tile([P, HIDDEN], F32, tag="xr", name=f"xr{mt}")
        nc.sync.dma_start(xr[:], x[ts(mt, P), :])
        xt = xtp.tile([P, NBLK, P], MAIN_DT, tag="xt", name=f"xt{mt}")
        if in_a:
            tpg = _PS_A[0].tile([P, NBLK, P], F32, tag="g")
            for k in range(NBLK):
                nc.tensor.transpose(tpg[:, k, :], xr[:, ts(k, P)], ident32[:])
        else:
            xb = xbp.tile([P, HIDDEN], MAIN_DT, tag="xb", name=f"xb{mt}")
            nc.scalar.activation(xb[:], xr[:], ACT_COPY, scale=1.0)
            tpg = ps_tp.tile([P, NBLK, P], MAIN_DT, tag="xtp",
                             name=f"xtp{mt}")
            for k in range(NBLK):
                nc.tensor.transpose(tpg[:, k, :], xb[:, ts(k, P)], identb[:])
        if mt % 2 == 0:
            cp(xt[:], tpg[:])
        else:
            nc.scalar.activation(xt[:], tpg[:], ACT_COPY, scale=1.0)
        return xt

    # ---- phase A+B scoped pools ----
    with (
        tc.tile_pool(name="nper", bufs=1) as nper,     # per-set persistents
        tc.tile_pool(name="nx", bufs=1) as nxp,        # per-set X iterates
        tc.tile_pool(name="nrot", bufs=2) as nrot,     # rotating temps
        tc.tile_pool(name="scr", bufs=1) as scr,       # polish/Q scratch
        tc.tile_pool(name="t1p", bufs=3) as t1p,       # Newton T1' ring
        tc.tile_pool(name="qpool", bufs=1) as qpool,
        tc.tile_pool(name="wstg", bufs=3) as wstg,
        tc.tile_pool(name="ps_g", bufs=3, space="PSUM") as ps_g,
    ):
        # ---------- S-prep (stage-wise in waves of 3 sets) ----------
        # Work with D = A - A^T in fp16 (S = D/2); scale factors folded into
        # the fused ops: P = (1+e)^2 I + 0.25 D^T D, X0 = aI + bP + cP^2,
        # c16 = eI12 - P (fp16 helper for the Q-stage B^T).
        stt = nc.vector.scalar_tensor_tensor
        MUL, ADD = mybir.AluOpType.mult, mybir.AluOpType.add
        d_s, p32_s, p16_s, x_s, c16_s = [], [], [], [], []
        aset_s, tpg_s, g_s, p2_s = {}, {}, {}, {}
        for s in range(NSETS):
            n0 = s * SETB
            aset = nrot.tile([P, SETB, P], F32, tag=f"a{s}", bufs=1)
            half = SETB // 2     # split across both HWDGE queues
            nc.sync.dma_start(aset[:, :half, :],
                              rmat[n0:n0 + half].rearrange("n p f -> p n f"))
            nc.scalar.dma_start(aset[:, half:, :],
                                rmat[n0 + half:n0 + SETB]
                                .rearrange("n p f -> p n f"))
            aset_s[s] = aset

        # W^T builder: PE transpose (fp32) -> evict fp16 straight into the
        # FT[og] tile that phase B later overwrites in place with Q^T W^T.
        wrows = {}

        def emit_wload(ot):
            wrow = wstg.tile([P, HIDDEN], F32, tag="wrow", name=f"w{ot}")
            nc.gpsimd.dma_start(wrow[:], w[ts(ot, P), :])
            wrows[ot] = wrow

        def emit_wtile(ot):
            """Transpose one 128-row W tile into FT[ot//4][:, :, ts(ot%4, P)]."""
            og, j4 = ot // 4, ot % 4
            wrow = wrows.pop(ot)
            tpg = ps_g.tile([P, NBLK, P], F32, tag="g")
            for k in range(NBLK):
                nc.tensor.transpose(tpg[:, k, :], wrow[:, ts(k, P)],
                                    ident32[:])
            dst = FT[og][:, :, ts(j4, P)]
            if ot % 2 == 0:
                nc.scalar.activation(dst, tpg[:], ACT_COPY, scale=1.0)
            else:
                cp(dst, tpg[:])

        for ot in range(3):
            emit_wload(ot)
        _PS_A.append(ps_g)
        xt_pre = [emit_xprep(0), emit_xprep(1)]

        for w0 in range(0, NSETS, 3):
            wave = range(w0, w0 + 3)
            for s in wave:
                tpg = ps_g.tile([P, SETB, P], F32, tag="g")
                for j in range(SETB):
                    nc.tensor.transpose(tpg[:, j, :], aset_s[s][:, j, :],
                                        ident32[:])
                tpg_s[s] = tpg
            for s in wave:
                dset = nper.tile([P, SETB, P], F32, tag=f"s{s}", name=f"d{s}")
                sub(dset[:], aset_s[s][:], tpg_s[s][:])  # D = A - A^T
                d_s.append(dset)
            emit_wtile(0)            # fills the PE while the DVE subs drain
            emit_wload(3)
            for s in wave:
                g = ps_g.tile([P, SETB, P], F32, tag="g")
                for j in range(SETB):                    # D^T D = -D^2
                    nc.tensor.matmul(g[:, j, :], lhsT=d_s[s][:, j, :],
                                     rhs=d_s[s][:, j, :], start=True,
                                     stop=True)
                g_s[s] = g
            emit_wtile(1)
            emit_wload(4)
            for s in wave:
                p32s = nper.tile([P, SETB, P], F32, tag=f"p32{s}",
                                 name=f"p32{s}")
                stt(p32s[:], g_s[s][:], 0.25, bc(eI2), MUL, ADD)
                p32_s.append(p32s)
            for s in wave:
                p16s = nper.tile([P, SETB, P], F16, tag=f"p16{s}",
                                 name=f"p16{s}")
                nc.scalar.activation(p16s[:], p32_s[s][:], ACT_COPY, scale=1.0)
                p16_s.append(p16s)
            emit_wtile(2)
            emit_wload(5)
            for s in wave:   # P^2 (fp16 matmul) for the degree-2 init
                g2p = ps_g.tile([P, SETB, P], F32, tag="g")
                for j in range(SETB):
                    nc.tensor.matmul(g2p[:, j, :], lhsT=p16_s[s][:, j, :],
                                     rhs=p16_s[s][:, j, :], start=True,
                                     stop=True)
                p2_s[s] = g2p
            emit_wtile(3)
            emit_wload(6)
            for s in wave:
                x0t = nxp.tile([P, SETB, P], F16, tag=f"xs{s}",
                               name=f"x0t{s}")
                stt(x0t[:], p32_s[s][:], float(X0_B), bc(aI0), MUL, ADD)
                xset = nxp.tile([P, SETB, P], F16, tag=f"x{s}",
                                name=f"x{s}_init")
                stt(xset[:], p2_s[s][:], float(X0_C), x0t[:], MUL, ADD)
                x_s.append(xset)
            for s in wave:   # c32 = eI12 - P (off the Newton critical path)
                c32 = aset_s[s]  # reuses the dead aset buffer
                stt(c32[:], p32_s[s][:], -1.0, bc(eI12), MUL, ADD)
                c16_s.append(c32)



        # ---------- Newton-Schulz fp16 iterations (stage-wise) ----------
        # per round: all g1 = P X; then per set either
        #   DVE path  (s odd):  T1' = 2I - g1 (DVE); g2 = X T1'   (8 mm)
        #   Act path  (s even): T1n = -g1 (scalar); g2 = X 2I + X T1n (12 mm)
        # so the PSUM-read elementwise load splits across both engines.
        for i in range(NEWTON_F16):
            g1_s, t1_s, g2_s = {}, {}, {}
            for s in range(NSETS):
                g1 = ps_g.tile([P, SETB, P], F32, tag="g")
                for j in range(SETB):
                    nc.tensor.matmul(g1[:, j, :], lhsT=p16_s[s][:, j, :],
                                     rhs=x_s[s][:, j, :], start=True, stop=True)
                g1_s[s] = g1
            for s in range(NSETS):
                t1n = t1p.tile([P, SETB, P], F16, tag="t1n")
                if (s + i) % 2 == 0:
                    nc.scalar.activation(t1n[:], g1_s[s][:], ACT_COPY,
                                         scale=-1.0)     # -P X
                else:
                    sub(t1n[:], bc(two_eye16), g1_s[s][:])  # 2I - P X
                t1_s[s] = t1n
            for s in range(NSETS):
                g2 = ps_g.tile([P, SETB, P], F32, tag="g")
                if (s + i) % 2 == 0:
                    for j in range(SETB):
                        nc.tensor.matmul(g2[:, j, :], lhsT=x_s[s][:, j, :],
                                         rhs=two_eye16[:], start=True,
                                         stop=False)
                        nc.tensor.matmul(g2[:, j, :], lhsT=x_s[s][:, j, :],
                                         rhs=t1_s[s][:, j, :], start=False,
                                         stop=True)
                else:
                    for j in range(SETB):
                        nc.tensor.matmul(g2[:, j, :], lhsT=x_s[s][:, j, :],
                                         rhs=t1_s[s][:, j, :], start=True,
                                         stop=True)
                g2_s[s] = g2
            do_sym = i in SYM_ITERS
            scale = 0.5 if do_sym else 1.0
            xnew = {}
            for s in range(NSETS):
                xset = nxp.tile([P, SETB, P], F16, tag=f"x{s}",
                                name=f"x{s}_{i}")
                if (s + i) % 2 == 1:
                    nc.scalar.activation(xset[:], g2_s[s][:], ACT_COPY,
                                         scale=scale)
                elif do_sym:
                    nc.vector.tensor_scalar(xset[:], g2_s[s][:], 0.5, None,
                                            mybir.AluOpType.mult)
                else:
                    cp(xset[:], g2_s[s][:])
                xnew[s] = xset
            if do_sym:
                tp_s = {}
                for s in range(NSETS):
                    tpg = ps_tp.tile([P, NBLK, P], F16, tag="xtp")
                    for j in range(SETB):
                        nc.tensor.transpose(tpg[:, j, :], xnew[s][:, j, :],
                                            identb[:])
                    tp_s[s] = tpg
                for s in range(NSETS):
                    xsym = nxp.tile([P, SETB, P], F16, tag=f"xs{s}",
                                    name=f"x{s}_{i}s")
                    add(xsym[:], xnew[s][:], tp_s[s][:, :SETB, :])
                    xnew[s] = xsym
            for s in range(NSETS):
                x_s[s] = xnew[s]
            # fill spare PE slots with W transposes (3-4 row-tiles per round)
            # and one x prefetch tile
            starts = [4, 8, 11, 14, 17, 20, 24]
            for ot in range(starts[i], starts[i + 1]):
                emit_wtile(ot)
                if ot + 3 < OUT_DIM // P:
                    emit_wload(ot + 3)
            xt_pre.append(emit_xprep(2 + i))

        # ---------- fp32 polish (stage-wise in waves of 3) ----------
        # final iterate evicted as fp16 so the Q matmul runs at 1 cyc/row.
        xf_s = []
        for s in range(NSETS):
            xf = nxp.tile([P, SETB, P], F32, tag=f"xf{s}", name=f"xf{s}_init")
            if s % 2 == 0:
                nc.scalar.activation(xf[:], x_s[s][:], ACT_COPY, scale=1.0)
            else:
                cp(xf[:], x_s[s][:])
            xf_s.append(xf)
        x16_s = [None] * NSETS
        for w0 in range(0, NSETS, 3):
            wave = range(w0, w0 + 3)
            g1_s, uf_s, g2_s = {}, {}, {}
            for s in wave:
                g1 = ps_g.tile([P, SETB, P], F32, tag="g")
                for j in range(SETB):
                    nc.tensor.matmul(g1[:, j, :], lhsT=p32_s[s][:, j, :],
                                     rhs=xf_s[s][:, j, :], start=True,
                                     stop=True)
                g1_s[s] = g1
            for s in wave:
                uf = scr.tile([P, SETB, P], F32, tag=f"sc{s % 3}")
                sub(uf[:], bc(twoI), g1_s[s][:])
                uf_s[s] = uf
            for s in wave:
                g2 = ps_g.tile([P, SETB, P], F32, tag="g")
                for j in range(SETB):
                    nc.tensor.matmul(g2[:, j, :], lhsT=xf_s[s][:, j, :],
                                     rhs=uf_s[s][:, j, :], start=True,
                                     stop=True)
                g2_s[s] = g2
            for s in wave:
                xf = nxp.tile([P, SETB, P], F32, tag=f"xf{s}",
                              name=f"xpol{s}")
                if s % 2 == 0:
                    nc.scalar.activation(xf[:], g2_s[s][:], ACT_COPY,
                                         scale=1.0)
                else:
                    cp(xf[:], g2_s[s][:])
                x16_s[s] = xf

        # Q = B @ X with B^T = (2+e)S + (eI12 - P) = 0.5(2+e) D + c32.
        # Full fp32: bt @ X has ~1e3x cancellation, fp16 here costs 7e-3.
        # Per set (= one of q/k/v): Q then immediately its two filt groups,
        # so FT[0..1] complete ~2 set-times before FT[4..5] and phase C's
        # early m-tiles never wait on trailing filt evictions.
        q_s = {}

        def q_lhsT(n):
            return q_s[n // SETB][:, n % SETB, :]

        for s in range(NSETS):
            bt = scr.tile([P, SETB, P], F32, tag=f"sc{s % 3}")
            stt(bt[:], d_s[s][:], float(0.5 * (2.0 + EPS)), c16_s[s][:],
                MUL, ADD)
            g = ps_g.tile([P, SETB, P], F32, tag="g")
            for j in range(SETB):
                nc.tensor.matmul(g[:, j, :], lhsT=bt[:, j, :],
                                 rhs=x16_s[s][:, j, :], start=True,
                                 stop=True)
            qset = qpool.tile([P, SETB, P], MAIN_DT, tag=f"q{s}",
                              name=f"q{s}")
            if s % 2 == 0:
                nc.scalar.activation(qset[:], g[:], ACT_COPY, scale=1.0)
            else:
                cp(qset[:], g[:])
            q_s[s] = qset
            # ---------- Phase B for this part: FT[og] <- Q^T @ FT[og] ----
            for og in (2 * s, 2 * s + 1):
                for kk in range(NBLK // 2):
                    fg = ps_g.tile([P, 2, 512], F32, tag="g")
                    for h in range(2):
                        k = 2 * kk + h
                        nc.tensor.matmul(fg[:, h, :],
                                         lhsT=q_lhsT(s * NBLK + k),
                                         rhs=FT[og][:, k, :], start=True,
                                         stop=True)
                    if kk == 3:
                        nc.scalar.activation(FT[og][:, ts(kk, 2), :], fg[:],
                                             ACT_COPY, scale=1.0)
                    else:
                        cp(FT[og][:, ts(kk, 2), :], fg[:])
            if s < 2:
                xt_pre.append(emit_xprep(8 + s))

        _CACHE["xt_pre"] = xt_pre

    # ---------- Phase C: matmul stream at the 16-bit PE roofline ----------
    xt_pre = _CACHE.pop("xt_pre")
    with tc.tile_pool(name="ps_out", bufs=6, space="PSUM") as ps_out:
        for mt in range(M_TILES):
            la = mt + LOOKAHEAD
            if PREFETCH <= la < M_TILES:
                xt_pre.append(emit_xprep(la))
            xt = xt_pre[mt]
            for og in range(O_TILES):
                po = ps_out.tile([P, 512], F32, tag="po",
                                 name=f"po{mt}_{og}")
                for k in range(NBLK):
                    nc.tensor.matmul(po[:], lhsT=xt[:, k, :],
                                     rhs=FT[og][:, k, :],
                                     start=(k == 0), stop=(k == NBLK - 1))
                ob = obp.tile([P, 512], F32, tag="ob", name=f"ob{mt}_{og}")
                add(ob[:], po[:], bias_bc[:, ts(og, 512)])
                eng = nc.sync if og % 2 == 0 else nc.scalar
                eng.dma_start(out[ts(mt, P), ts(og, 512)], ob[:])


_CACHE = {}


def build():
    if "nc" in _CACHE:
        return _CACHE["nc"]
    import contextlib

    nc = bacc.Bacc("TRN2", target_bir_lowering=False, debug=False)
    with tile.TileContext(nc) as tc:
        with contextlib.ExitStack() as ctx:
            build_body(ctx, tc)
    nc.compile()
    _CACHE["nc"] = nc
    return nc


def make_in_maps(attn_weight, bias, x, q_R, k_R, v_R):
    rmat = np.ascontiguousarray(
        np.concatenate([q_R, k_R, v_R], axis=0), dtype=np.float32)
    w = np.ascontiguousarray(attn_weight, dtype=np.float32)
    b = np.ascontiguousarray(bias, dtype=np.float32)
    return [
        {"x": np.ascontiguousarray(x[c], dtype=np.float32),
         "w": w, "bias": b, "rmat": rmat}
        for c in range(N_CORES)
    ]


def kernel(attn_weight, bias, x, q_R, k_R, v_R, **run_kwargs):
    nc = build()
    in_maps = make_in_maps(attn_weight, bias, x, q_R, k_R, v_R)
    res = run_bass_kernel_spmd(nc, in_maps, core_ids=list(range(N_CORES)),
                               **run_kwargs)
    out = np.stack([res.results[c]["out"] for c in range(N_CORES)], axis=0)
    _CACHE["last_results"] = res
    return out
